# revision 25
# baseline (speedup 1.0000x reference)
"""GNN message passing (gather + segment-sum) on 8 Trainium2 NeuronCores.

Strategy (node-range sharding per the spec's sharding_hint):
  - Destination nodes are range-sharded across the 8 cores (12500 nodes
    each), so each core owns a disjoint slice of the output and no
    cross-core reduction is needed.
  - The device-side gather uses the batched SWDGE row-gather
    (`nc.gpsimd.dma_gather`).  Its indices are int16, so x is packed as
    [25001, 256] (4 node rows per packed row plus one zero row) and source
    nodes are split into 4 "colors" by src % 4; color q gathers from
    column slice q*64:(q+1)*64 with elem_step=256 and index src//4.
  - Per core and color, the core's nodes are sorted by color-in-degree
    (descending) and grouped into 98 tiles of 128 (one SBUF partition per
    node).  Because tiles are degree-sorted, the set of tiles still
    needing accumulation pass k is a prefix [0, n_k): pass k gathers the
    k-th color-q source row of nodes in tiles 0..n_k (dummy index -> zero
    row past a node's degree) and ONE wide vector add folds the whole
    pass into the color's persistent [128, 98*64] SBUF accumulator.
  - The per-color slot stream (pass-major) is chunked into gather calls
    of CALL_TILES tiles (CALL_TILES*128 idxs — 1024 is the hard max per
    self-triggered call, set by the 1024-descriptor SWDGE ring) rotating
    over the 4 SWDGE queues, writing into a ring of window buffers; adds
    read per-pass segments of those windows.
  - Engine placement avoids the DVE<->GpSimd shared-SBUF-port lock: the
    pass-0 copies run on the ACT engine (a DVE fp32 tensor_copy engages
    2-port perf mode and starves SWDGE descriptor generation); the k>0
    adds are DVE fp32 tensor_adds, which only support 1-port mode.
  - Accumulators are fp16 (tolerance is 2e-2; halves store traffic on
    the saturated DMA path) and tile-suffixes stream to DRAM as soon as
    their last pass has run.  Idx tables load in per-color chunks under
    the startup window; a warmup gather pays the one-time Q7 ext-isa
    IRAM load during that window.
  - The host undoes the four degree-sort permutations, sums the color
    partials in fp32, and concatenates the 8 node-range slices.
"""

import numpy as np
from contextlib import ExitStack

import concourse.bacc as bacc
import concourse.bass as bass
import concourse.tile as tile
import concourse.mybir as mybir
from concourse.bass_utils import run_bass_kernel_spmd

N_NODES = 100000
N_EDGES = 1250000
D = 64
N_CORES = 8
NPC = N_NODES // N_CORES          # 12500 nodes per core
P = 128
TILES = (NPC + P - 1) // P        # 98 node tiles per core
NPC_PAD = TILES * P               # 12544
COLORS = 4
RPACK = N_NODES // COLORS + 1     # 25001 packed rows (last = zeros)
DUMMY = RPACK - 1

CALL_TILES = 16                   # tiles (x128 idxs) per dma_gather call
WIN = 32                          # slots per window buffer
WIN_BUFS = 6                      # window ring depth
SCRATCH = 16384                   # SWDGE descriptor-ring carveout (B/partition)

# Set by test.py for profiling; harness path leaves these untouched.
PROFILE = False
TRACE_CORES = None
LAST_EXEC_NS = None
LAST_RESULTS = None

_COMPILE_CACHE = {}


def _schedule(K):
    """Shared (core-independent) slot stream + call/add/window schedule.

    K: [COLORS][TILES] non-increasing per-tile max color-degree.
    Returns per-color dicts with:
      n0        — #tiles with K>0
      passes    — list of (k, n_k, slot_lo)
      calls     — list of (slot_lo, n_slots, idx_col_off)  (idx cols global)
      col_off   — global idx column offset of the color's slot 0
    Slot s of pass k targets tile (s - slot_lo); gather rows for slot s sit
    at idx cols [col_off + s*8, col_off + s*8 + 8).
    """
    sched = []
    col_off = 0
    for q in range(COLORS):
        Kq = K[q]
        kmax = max(Kq)
        n0 = sum(1 for v in Kq if v > 0)
        passes = []
        slot = 0
        for k in range(kmax):
            n_k = sum(1 for v in Kq if v > k)
            passes.append((k, n_k, slot))
            slot += n_k
        n_slots = slot
        calls = []
        for w_lo in range(0, n_slots, WIN):
            w_hi = min(w_lo + WIN, n_slots)
            for c_lo in range(w_lo, w_hi, CALL_TILES):
                c_n = min(CALL_TILES, w_hi - c_lo)
                calls.append((c_lo, c_n, col_off + c_lo * 8))
        sched.append(dict(n0=n0, passes=passes, calls=calls,
                          n_slots=n_slots, col_off=col_off))
        col_off += n_slots * 8
    return sched, col_off


def _preprocess(edge_index, x):
    """Host-side sharding: per-core padded gather-index tables."""
    dest = np.asarray(edge_index[0]).astype(np.int64)
    src = np.asarray(edge_index[1]).astype(np.int64)
    x = np.ascontiguousarray(np.asarray(x), dtype=np.float32)

    x_pack = np.zeros((RPACK, COLORS * D), np.float32)
    x_pack[:N_NODES // COLORS] = x.reshape(N_NODES // COLORS, COLORS * D)

    core_of = dest // NPC
    # per (core, color): (perm, deg_pad, starts_pad, srcs_sorted)
    pc = [[None] * COLORS for _ in range(N_CORES)]
    K_all = np.zeros((N_CORES, COLORS, TILES), np.int64)
    for c in range(N_CORES):
        m = core_of == c
        d_loc = dest[m] - c * NPC
        s_c = src[m]
        color = s_c % COLORS
        for q in range(COLORS):
            mq = color == q
            d_q = d_loc[mq]
            s_q = (s_c[mq] // COLORS).astype(np.int16)
            deg = np.bincount(d_q, minlength=NPC)
            order = np.argsort(d_q, kind="stable")
            s_sorted = s_q[order]
            starts = np.zeros(NPC, np.int64)
            starts[1:] = np.cumsum(deg)[:-1]
            perm = np.argsort(-deg, kind="stable")
            deg_pad = np.concatenate([deg[perm],
                                      np.zeros(NPC_PAD - NPC, np.int64)])
            starts_pad = np.concatenate([starts[perm],
                                         np.zeros(NPC_PAD - NPC, np.int64)])
            K_all[c, q] = deg_pad.reshape(TILES, P)[:, 0]
            pc[c][q] = (perm, deg_pad, starts_pad, s_sorted)

    K = K_all.max(axis=0)                      # [COLORS, TILES] shared
    K_key = tuple(tuple(int(v) for v in K[q]) for q in range(COLORS))
    sched, tot_cols = _schedule(K_key)

    idx_maps = []
    for c in range(N_CORES):
        cols = np.full((tot_cols, 16), DUMMY, np.int16)  # [cols, 16] pre-wrap
        for q in range(COLORS):
            perm, deg_pad, starts_pad, s_sorted = pc[c][q]
            s_safe = np.concatenate([s_sorted, np.zeros(1, np.int16)])
            sc = sched[q]
            for k, n_k, slot_lo in sc["passes"]:
                r = np.arange(n_k * P)         # node ranks of this pass
                dg = deg_pad[r]
                st = starts_pad[r]
                pos = np.minimum(st + k, len(s_safe) - 1)
                vals = np.where(k < dg, s_safe[pos], DUMMY)   # [n_k*128]
                lo = sc["col_off"] + slot_lo * 8
                cols[lo:lo + n_k * 8] = vals.reshape(n_k * 8, 16)
        wrapped = cols.T                        # [16, tot_cols]
        idx_maps.append(np.ascontiguousarray(
            np.tile(wrapped, (8, 1))))          # [128, tot_cols]

    perms = [[pc[c][q][0] for q in range(COLORS)] for c in range(N_CORES)]
    return x_pack, idx_maps, perms, K_key, sched, tot_cols


def _build_program(K, sched, tot_cols):
    nc = bacc.Bacc("TRN2", target_bir_lowering=False, debug=False,
                   num_devices=N_CORES, num_swdge_queues=4,
                   dynamic_dma_scratch_size=SCRATCH)
    x_dram = nc.dram_tensor("x", [RPACK, COLORS * D], mybir.dt.float32,
                            kind="ExternalInput")
    idx_dram = nc.dram_tensor("idx", [P, tot_cols], mybir.dt.int16,
                              kind="ExternalInput")
    out_dram = nc.dram_tensor("out", [COLORS, P, TILES * D], mybir.dt.float16,
                              kind="ExternalOutput")

    with tile.TileContext(nc) as tc, ExitStack() as ctx:
        idx_pool = ctx.enter_context(tc.tile_pool(name="idx", bufs=1))
        g_pool = ctx.enter_context(tc.tile_pool(name="g", bufs=WIN_BUFS))
        acc_pool = ctx.enter_context(tc.tile_pool(name="acc", bufs=2))

        idx_sb = idx_pool.tile([P, tot_cols], mybir.dt.int16, tag="idx",
                               name="idx_sb")

        # Warmup gathers (row 0 -> scratch), one per SWDGE queue: pays each
        # Q7 core pair's one-time ext-isa first-call cost while the idx
        # table DMA is still in flight.
        warm_idx = idx_pool.tile([P, 8], mybir.dt.int16, tag="warm_idx",
                                 name="warm_idx")
        nc.vector.memset(warm_idx[:], 0.0)
        warm_g = g_pool.tile([P, 4 * D], mybir.dt.float32, tag="warm_g",
                             name="warm_g", bufs=1)
        wgv = warm_g[:].rearrange("p (s d) -> p s d", d=D)
        for wq in range(1):
            nc.gpsimd.dma_gather(
                out_ap=wgv[:, wq:wq + 1, :],
                in_ap=x_dram.ap()[:, 0:D],
                idxs_ap=warm_idx[:],
                num_idxs=P, num_idxs_reg=P,
                elem_size=D, elem_step=COLORS * D, queue_num=wq)

        # Hoisted num_idxs registers (one MOVE per distinct size instead of
        # one per gather call).
        regs = {}

        def nreg(n):
            if n not in regs:
                regs[n] = nc.gpsimd.to_reg(n)
            return regs[n]

        # Per-color idx loads (color 0 split again) so the first gather only
        # waits for a small slice instead of the whole 2.6 MB table.
        idx_chunks = []
        first_cut = min(WIN * 8, sched[0]["n_slots"] * 8)
        idx_chunks.append((0, first_cut))
        idx_chunks.append((first_cut, sched[0]["n_slots"] * 8))
        for q in range(1, COLORS):
            sc = sched[q]
            idx_chunks.append((sc["col_off"], sc["col_off"] + sc["n_slots"] * 8))
        for lo, hi in idx_chunks:
            if hi > lo:
                nc.sync.dma_start(out=idx_sb[:, lo:hi],
                                  in_=idx_dram.ap()[:, lo:hi])

        qn = 0
        for q in range(COLORS):
            sc = sched[q]
            n_slots = sc["n_slots"]
            acc = acc_pool.tile([P, TILES * D], mybir.dt.float16,
                                tag="acc", name=f"acc{q}")
            n_win = (n_slots + WIN - 1) // WIN
            wins = []
            for w in range(n_win):
                wins.append(g_pool.tile([P, WIN * D], mybir.dt.float32,
                                        tag="g", name=f"g{q}_{w}"))
            # gather calls: chunk the slot stream
            for c_lo, c_n, col in sc["calls"]:
                w = c_lo // WIN
                lo_in_w = c_lo - w * WIN
                gv = wins[w][:].rearrange("p (s d) -> p s d", d=D)
                nc.gpsimd.dma_gather(
                    out_ap=gv[:, lo_in_w:lo_in_w + c_n, :],
                    in_ap=x_dram.ap()[:, q * D:(q + 1) * D],
                    idxs_ap=idx_sb[:, col:col + c_n * 8],
                    num_idxs=c_n * P,
                    num_idxs_reg=nreg(c_n * P),
                    elem_size=D,
                    elem_step=COLORS * D,
                    queue_num=qn % 4,
                )
                qn += 1
            # adds: per pass, split at window boundaries; suffix tiles whose
            # last pass just ran are streamed out as soon as they are final.
            n_next = {k: (sc["passes"][i + 1][1] if i + 1 < len(sc["passes"])
                          else 0)
                      for i, (k, _, _) in enumerate(sc["passes"])}
            pend_hi = sc["n0"]            # final-region [pend_lo, pend_hi)
            for k, n_k, slot_lo in sc["passes"]:
                s_lo, s_hi = slot_lo, slot_lo + n_k
                seg = s_lo
                while seg < s_hi:
                    w = seg // WIN
                    seg_hi = min(s_hi, (w + 1) * WIN)
                    src_ap = wins[w][:, (seg - w * WIN) * D:
                                     (seg_hi - w * WIN) * D]
                    dst_ap = acc[:, (seg - s_lo) * D:(seg_hi - s_lo) * D]
                    if k == 0:
                        # ACT-engine copy: a DVE fp32 tensor_copy runs in
                        # 2-port perf mode and locks GpSimd (SWDGE descriptor
                        # generation) out of the shared SBUF port pair.
                        nc.scalar.copy(dst_ap, src_ap)
                    else:
                        nc.vector.tensor_add(dst_ap, dst_ap, src_ap)
                    seg = seg_hi
                pend_lo = n_next[k]       # tiles [pend_lo, n_k) now final
                if pend_hi - pend_lo >= 16 or pend_lo == 0:
                    nc.sync.dma_start(
                        out=out_dram.ap()[q][:, pend_lo * D:pend_hi * D],
                        in_=acc[:, pend_lo * D:pend_hi * D])
                    pend_hi = pend_lo
            if sc["n0"] < TILES:
                nc.vector.memset(acc[:, sc["n0"] * D:], 0.0)
                nc.sync.dma_start(out=out_dram.ap()[q][:, sc["n0"] * D:],
                                  in_=acc[:, sc["n0"] * D:])
    nc.compile()
    return nc


def _install_profile_shim():
    """trace=True under axon needs the NTFF hook that this image's antenv
    lacks; register the ctypes-based one from trn_agent_boot."""
    import sys, types
    import concourse.bass_utils as bu
    if "antenv.axon_hooks" not in sys.modules:
        from trn_agent_boot.trn_boot import _ntff_profile_via_ctypes
        shim = types.ModuleType("antenv.axon_hooks")
        hook = _ntff_profile_via_ctypes("/opt/axon/libaxon_pjrt.so")
        shim.get_axon_ntff_profile_hook = lambda: hook
        shim.set_axon_ntff_profile_hook = lambda h: None
        sys.modules["antenv.axon_hooks"] = shim
    bu.upload_artifacts = lambda tmpdir: f"local:{tmpdir}"


def kernel(edge_index, x):
    global LAST_EXEC_NS, LAST_RESULTS
    x_pack, idx_maps, perms, K, sched, tot_cols = _preprocess(edge_index, x)

    cache_key = (K, tot_cols)
    if cache_key not in _COMPILE_CACHE:
        _COMPILE_CACHE[cache_key] = _build_program(K, sched, tot_cols)
    nc = _COMPILE_CACHE[cache_key]

    in_maps = [{"x": x_pack, "idx": idx_maps[c]} for c in range(N_CORES)]
    kwargs = {}
    if PROFILE:
        _install_profile_shim()
        kwargs = dict(trace=True, trace_cores=TRACE_CORES)
    res = run_bass_kernel_spmd(nc, in_maps, core_ids=list(range(N_CORES)),
                               **kwargs)
    LAST_EXEC_NS = res.exec_time_ns
    LAST_RESULTS = res

    out = np.empty((N_NODES, D), np.float32)
    for c in range(N_CORES):
        dev = res.results[c]["out"]            # [COLORS, 128, TILES*D] fp16
        sl = np.zeros((NPC, D), np.float32)
        for q in range(COLORS):
            a = dev[q].astype(np.float32).reshape(P, TILES, D)
            a = a.transpose(1, 0, 2).reshape(NPC_PAD, D)[:NPC]
            tmp = np.empty((NPC, D), np.float32)
            tmp[perms[c][q]] = a
            sl += tmp
        out[c * NPC:(c + 1) * NPC] = sl
    return out


# revision 26
# speedup vs baseline: 1.0313x; 1.0313x over previous
"""GNN message passing (gather + segment-sum) on 8 Trainium2 NeuronCores.

Strategy (node-range sharding per the spec's sharding_hint):
  - Destination nodes are range-sharded across the 8 cores (12500 nodes
    each), so each core owns a disjoint slice of the output and no
    cross-core reduction is needed.
  - The device-side gather uses the batched SWDGE row-gather
    (`nc.gpsimd.dma_gather`).  Its indices are int16, so x is packed as
    [25001, 256] (4 node rows per packed row plus one zero row) and source
    nodes are split into 4 "colors" by src % 4; color q gathers from
    column slice q*64:(q+1)*64 with elem_step=256 and index src//4.
  - Per core and color, the core's nodes are sorted by color-in-degree
    (descending) and grouped into 98 tiles of 128 (one SBUF partition per
    node).  Because tiles are degree-sorted, the set of tiles still
    needing accumulation pass k is a prefix [0, n_k): pass k gathers the
    k-th color-q source row of nodes in tiles 0..n_k (dummy index -> zero
    row past a node's degree) and ONE wide vector add folds the whole
    pass into the color's persistent [128, 98*64] SBUF accumulator.
  - The per-color slot stream (pass-major) is chunked into gather calls
    of CALL_TILES tiles (CALL_TILES*128 idxs — 1024 is the hard max per
    self-triggered call, set by the 1024-descriptor SWDGE ring) rotating
    over the 4 SWDGE queues, writing into a ring of window buffers; adds
    read per-pass segments of those windows.
  - Engine placement avoids the DVE<->GpSimd shared-SBUF-port lock: the
    pass-0 copies run on the ACT engine (a DVE fp32 tensor_copy engages
    2-port perf mode and starves SWDGE descriptor generation); the k>0
    adds are DVE fp32 tensor_adds, which only support 1-port mode.
  - Accumulators are fp16 (tolerance is 2e-2; halves store traffic on
    the saturated DMA path) and tile-suffixes stream to DRAM as soon as
    their last pass has run.  Idx tables load in per-color chunks under
    the startup window; a warmup gather pays the one-time Q7 ext-isa
    IRAM load during that window.
  - The host undoes the four degree-sort permutations, sums the color
    partials in fp32, and concatenates the 8 node-range slices.
"""

import numpy as np
from contextlib import ExitStack

import concourse.bacc as bacc
import concourse.bass as bass
import concourse.tile as tile
import concourse.mybir as mybir
from concourse.bass_utils import run_bass_kernel_spmd

N_NODES = 100000
N_EDGES = 1250000
D = 64
N_CORES = 8
NPC = N_NODES // N_CORES          # 12500 nodes per core
P = 128
TILES = (NPC + P - 1) // P        # 98 node tiles per core
NPC_PAD = TILES * P               # 12544
COLORS = 4
RPACK = N_NODES // COLORS + 1     # 25001 packed rows (last = zeros)
DUMMY = RPACK - 1

CALL_TILES = 16                   # tiles (x128 idxs) per dma_gather call
WIN = 32                          # slots per window buffer
WIN_BUFS = 8                      # window ring depth
SCRATCH = 16384                   # SWDGE descriptor-ring carveout (B/partition)

# Set by test.py for profiling; harness path leaves these untouched.
PROFILE = False
TRACE_CORES = None
LAST_EXEC_NS = None
LAST_RESULTS = None

_COMPILE_CACHE = {}


def _schedule(K):
    """Shared (core-independent) slot stream + call/add/window schedule.

    K: [COLORS][TILES] non-increasing per-tile max color-degree.
    Returns per-color dicts with:
      n0        — #tiles with K>0
      passes    — list of (k, n_k, slot_lo)
      calls     — list of (slot_lo, n_slots, idx_col_off)  (idx cols global)
      col_off   — global idx column offset of the color's slot 0
    Slot s of pass k targets tile (s - slot_lo); gather rows for slot s sit
    at idx cols [col_off + s*8, col_off + s*8 + 8).
    """
    sched = []
    col_off = 0
    for q in range(COLORS):
        Kq = K[q]
        kmax = max(Kq)
        n0 = sum(1 for v in Kq if v > 0)
        passes = []
        slot = 0
        for k in range(kmax):
            n_k = sum(1 for v in Kq if v > k)
            passes.append((k, n_k, slot))
            slot += n_k
        n_slots = slot
        calls = []
        for w_lo in range(0, n_slots, WIN):
            w_hi = min(w_lo + WIN, n_slots)
            for c_lo in range(w_lo, w_hi, CALL_TILES):
                c_n = min(CALL_TILES, w_hi - c_lo)
                calls.append((c_lo, c_n, col_off + c_lo * 8))
        sched.append(dict(n0=n0, passes=passes, calls=calls,
                          n_slots=n_slots, col_off=col_off))
        col_off += n_slots * 8
    return sched, col_off


def _preprocess(edge_index, x):
    """Host-side sharding: per-core padded gather-index tables."""
    dest = np.asarray(edge_index[0]).astype(np.int64)
    src = np.asarray(edge_index[1]).astype(np.int64)
    x = np.ascontiguousarray(np.asarray(x), dtype=np.float32)

    x_pack = np.zeros((RPACK, COLORS * D), np.float32)
    x_pack[:N_NODES // COLORS] = x.reshape(N_NODES // COLORS, COLORS * D)

    core_of = dest // NPC
    # per (core, color): (perm, deg_pad, starts_pad, srcs_sorted)
    pc = [[None] * COLORS for _ in range(N_CORES)]
    K_all = np.zeros((N_CORES, COLORS, TILES), np.int64)
    for c in range(N_CORES):
        m = core_of == c
        d_loc = dest[m] - c * NPC
        s_c = src[m]
        color = s_c % COLORS
        for q in range(COLORS):
            mq = color == q
            d_q = d_loc[mq]
            s_q = (s_c[mq] // COLORS).astype(np.int16)
            deg = np.bincount(d_q, minlength=NPC)
            order = np.argsort(d_q, kind="stable")
            s_sorted = s_q[order]
            starts = np.zeros(NPC, np.int64)
            starts[1:] = np.cumsum(deg)[:-1]
            perm = np.argsort(-deg, kind="stable")
            deg_pad = np.concatenate([deg[perm],
                                      np.zeros(NPC_PAD - NPC, np.int64)])
            starts_pad = np.concatenate([starts[perm],
                                         np.zeros(NPC_PAD - NPC, np.int64)])
            K_all[c, q] = deg_pad.reshape(TILES, P)[:, 0]
            pc[c][q] = (perm, deg_pad, starts_pad, s_sorted)

    K = K_all.max(axis=0)                      # [COLORS, TILES] shared
    K_key = tuple(tuple(int(v) for v in K[q]) for q in range(COLORS))
    sched, tot_cols = _schedule(K_key)

    idx_maps = []
    for c in range(N_CORES):
        cols = np.full((tot_cols, 16), DUMMY, np.int16)  # [cols, 16] pre-wrap
        for q in range(COLORS):
            perm, deg_pad, starts_pad, s_sorted = pc[c][q]
            s_safe = np.concatenate([s_sorted, np.zeros(1, np.int16)])
            sc = sched[q]
            for k, n_k, slot_lo in sc["passes"]:
                r = np.arange(n_k * P)         # node ranks of this pass
                dg = deg_pad[r]
                st = starts_pad[r]
                pos = np.minimum(st + k, len(s_safe) - 1)
                vals = np.where(k < dg, s_safe[pos], DUMMY)   # [n_k*128]
                lo = sc["col_off"] + slot_lo * 8
                cols[lo:lo + n_k * 8] = vals.reshape(n_k * 8, 16)
        wrapped = cols.T                        # [16, tot_cols]
        idx_maps.append(np.ascontiguousarray(
            np.tile(wrapped, (8, 1))))          # [128, tot_cols]

    perms = [[pc[c][q][0] for q in range(COLORS)] for c in range(N_CORES)]
    return x_pack, idx_maps, perms, K_key, sched, tot_cols


def _build_program(K, sched, tot_cols):
    nc = bacc.Bacc("TRN2", target_bir_lowering=False, debug=False,
                   num_devices=N_CORES, num_swdge_queues=4,
                   dynamic_dma_scratch_size=SCRATCH)
    x_dram = nc.dram_tensor("x", [RPACK, COLORS * D], mybir.dt.float32,
                            kind="ExternalInput")
    idx_dram = nc.dram_tensor("idx", [P, tot_cols], mybir.dt.int16,
                              kind="ExternalInput")
    out_dram = nc.dram_tensor("out", [COLORS, P, TILES * D], mybir.dt.float16,
                              kind="ExternalOutput")

    with tile.TileContext(nc) as tc, ExitStack() as ctx:
        idx_pool = ctx.enter_context(tc.tile_pool(name="idx", bufs=1))
        g_pool = ctx.enter_context(tc.tile_pool(name="g", bufs=WIN_BUFS))
        acc_pool = ctx.enter_context(tc.tile_pool(name="acc", bufs=2))

        idx_sb = idx_pool.tile([P, tot_cols], mybir.dt.int16, tag="idx",
                               name="idx_sb")

        # Warmup gathers (row 0 -> scratch), one per SWDGE queue: pays each
        # Q7 core pair's one-time ext-isa first-call cost while the idx
        # table DMA is still in flight.
        warm_idx = idx_pool.tile([P, 8], mybir.dt.int16, tag="warm_idx",
                                 name="warm_idx")
        nc.vector.memset(warm_idx[:], 0.0)
        warm_g = g_pool.tile([P, 4 * D], mybir.dt.float32, tag="warm_g",
                             name="warm_g", bufs=1)
        wgv = warm_g[:].rearrange("p (s d) -> p s d", d=D)
        for wq in range(1):
            nc.gpsimd.dma_gather(
                out_ap=wgv[:, wq:wq + 1, :],
                in_ap=x_dram.ap()[:, 0:D],
                idxs_ap=warm_idx[:],
                num_idxs=P, num_idxs_reg=P,
                elem_size=D, elem_step=COLORS * D, queue_num=wq)

        # Hoisted num_idxs registers (one MOVE per distinct size instead of
        # one per gather call).
        regs = {}

        def nreg(n):
            if n not in regs:
                regs[n] = nc.gpsimd.to_reg(n)
            return regs[n]

        # Per-color idx loads (color 0 split again) so the first gather only
        # waits for a small slice instead of the whole 2.6 MB table.
        idx_chunks = []
        first_cut = min(WIN * 8, sched[0]["n_slots"] * 8)
        idx_chunks.append((0, first_cut))
        idx_chunks.append((first_cut, sched[0]["n_slots"] * 8))
        for q in range(1, COLORS):
            sc = sched[q]
            idx_chunks.append((sc["col_off"], sc["col_off"] + sc["n_slots"] * 8))
        for lo, hi in idx_chunks:
            if hi > lo:
                nc.sync.dma_start(out=idx_sb[:, lo:hi],
                                  in_=idx_dram.ap()[:, lo:hi])

        qn = 0
        for q in range(COLORS):
            sc = sched[q]
            n_slots = sc["n_slots"]
            acc = acc_pool.tile([P, TILES * D], mybir.dt.float16,
                                tag="acc", name=f"acc{q}")
            n_win = (n_slots + WIN - 1) // WIN
            wins = []
            for w in range(n_win):
                wins.append(g_pool.tile([P, WIN * D], mybir.dt.float32,
                                        tag="g", name=f"g{q}_{w}"))
            # gather calls: chunk the slot stream
            for c_lo, c_n, col in sc["calls"]:
                w = c_lo // WIN
                lo_in_w = c_lo - w * WIN
                gv = wins[w][:].rearrange("p (s d) -> p s d", d=D)
                nc.gpsimd.dma_gather(
                    out_ap=gv[:, lo_in_w:lo_in_w + c_n, :],
                    in_ap=x_dram.ap()[:, q * D:(q + 1) * D],
                    idxs_ap=idx_sb[:, col:col + c_n * 8],
                    num_idxs=c_n * P,
                    num_idxs_reg=nreg(c_n * P),
                    elem_size=D,
                    elem_step=COLORS * D,
                    queue_num=qn % 4,
                )
                qn += 1
            # adds: per pass, split at window boundaries; suffix tiles whose
            # last pass just ran are streamed out as soon as they are final.
            n_next = {k: (sc["passes"][i + 1][1] if i + 1 < len(sc["passes"])
                          else 0)
                      for i, (k, _, _) in enumerate(sc["passes"])}
            pend_hi = sc["n0"]            # final-region [pend_lo, pend_hi)
            for k, n_k, slot_lo in sc["passes"]:
                s_lo, s_hi = slot_lo, slot_lo + n_k
                seg = s_lo
                while seg < s_hi:
                    w = seg // WIN
                    seg_hi = min(s_hi, (w + 1) * WIN)
                    src_ap = wins[w][:, (seg - w * WIN) * D:
                                     (seg_hi - w * WIN) * D]
                    dst_ap = acc[:, (seg - s_lo) * D:(seg_hi - s_lo) * D]
                    if k == 0:
                        # ACT-engine copy: a DVE fp32 tensor_copy runs in
                        # 2-port perf mode and locks GpSimd (SWDGE descriptor
                        # generation) out of the shared SBUF port pair.
                        nc.scalar.copy(dst_ap, src_ap)
                    else:
                        nc.vector.tensor_add(dst_ap, dst_ap, src_ap)
                    seg = seg_hi
                pend_lo = n_next[k]       # tiles [pend_lo, n_k) now final
                if pend_hi - pend_lo >= 16 or pend_lo == 0:
                    nc.sync.dma_start(
                        out=out_dram.ap()[q][:, pend_lo * D:pend_hi * D],
                        in_=acc[:, pend_lo * D:pend_hi * D])
                    pend_hi = pend_lo
            if sc["n0"] < TILES:
                nc.vector.memset(acc[:, sc["n0"] * D:], 0.0)
                nc.sync.dma_start(out=out_dram.ap()[q][:, sc["n0"] * D:],
                                  in_=acc[:, sc["n0"] * D:])
    nc.compile()
    return nc


def _install_profile_shim():
    """trace=True under axon needs the NTFF hook that this image's antenv
    lacks; register the ctypes-based one from trn_agent_boot."""
    import sys, types
    import concourse.bass_utils as bu
    if "antenv.axon_hooks" not in sys.modules:
        from trn_agent_boot.trn_boot import _ntff_profile_via_ctypes
        shim = types.ModuleType("antenv.axon_hooks")
        hook = _ntff_profile_via_ctypes("/opt/axon/libaxon_pjrt.so")
        shim.get_axon_ntff_profile_hook = lambda: hook
        shim.set_axon_ntff_profile_hook = lambda h: None
        sys.modules["antenv.axon_hooks"] = shim
    bu.upload_artifacts = lambda tmpdir: f"local:{tmpdir}"


def kernel(edge_index, x):
    global LAST_EXEC_NS, LAST_RESULTS
    x_pack, idx_maps, perms, K, sched, tot_cols = _preprocess(edge_index, x)

    cache_key = (K, tot_cols)
    if cache_key not in _COMPILE_CACHE:
        _COMPILE_CACHE[cache_key] = _build_program(K, sched, tot_cols)
    nc = _COMPILE_CACHE[cache_key]

    in_maps = [{"x": x_pack, "idx": idx_maps[c]} for c in range(N_CORES)]
    kwargs = {}
    if PROFILE:
        _install_profile_shim()
        kwargs = dict(trace=True, trace_cores=TRACE_CORES)
    res = run_bass_kernel_spmd(nc, in_maps, core_ids=list(range(N_CORES)),
                               **kwargs)
    LAST_EXEC_NS = res.exec_time_ns
    LAST_RESULTS = res

    out = np.empty((N_NODES, D), np.float32)
    for c in range(N_CORES):
        dev = res.results[c]["out"]            # [COLORS, 128, TILES*D] fp16
        sl = np.zeros((NPC, D), np.float32)
        for q in range(COLORS):
            a = dev[q].astype(np.float32).reshape(P, TILES, D)
            a = a.transpose(1, 0, 2).reshape(NPC_PAD, D)[:NPC]
            tmp = np.empty((NPC, D), np.float32)
            tmp[perms[c][q]] = a
            sl += tmp
        out[c * NPC:(c + 1) * NPC] = sl
    return out
